# revision 95
# baseline (speedup 1.0000x reference)
"""Attentional pooling layer on Trainium2 (Bass/Tile), 8-core batch-parallel.

Reference computation per batch b:
    scores[hw, n] = sum_c f[c, hw] * w[c, n]          (mm1)
    num           = softplus(scores)                  (relu/abs/Exp + quad poly)
    denom[n]      = sum_hw num[hw, n] + 16*CONST      (PE reduce, pad-row trick)
    att[hw, n]    = (num + CONST) / denom[n]          (DVE recip, PE bcast, DVE)
    out[c, n]     = sum_hw f[c, hw] * att[hw, n]      (mm2)

The kernel is DMA-bound (weights in + out out dominate), so both weights and
output travel as bf16 (tolerance is 2e-2; this lands ~4e-3), and all matmuls
take bf16 moving operands.  4 batches are packed per 128-partition group at
32-partition offsets 0/32/64/96 (PE tile_position); mm1 runs M=32 with
zero-padded feature columns so the 16 garbage rows per 32-block hold clean
zeros.  softplus(x) = max(x,0) + ln(1+exp(-|x|)); the ln(1+t) factor is a
minimax quadratic C1*t + C2*t^2 (max err 4.5e-3), so the only table-based
activation is Exp — a single table load for the whole kernel.  |x| is
2*relu(x) - x because an op may read PSUM only once; GPSIMD cannot touch PSUM
at all, so Pool evaluates the (all-SBUF) polynomial and num sum while the
PSUM->SBUF output copies rotate over ACT/DVE.  The denominator's +16*CONST
rides inside the bd reduction matmul: each block's 16 zero-pad rows carry
weight WPAD = CONST/poly(1) instead of 0.

The per-group tail (reduce/broadcast/att/mm2) is software-pipelined one group
behind mm1 so the in-order PE queue never waits on the softplus chain:
PE program is mm1(g, chunk0) | denom+bcast(g-1) | mm2(g-1) | mm1(g, 1-3).

32 batches per core = 8 groups of 4, no ragged tail.
"""

import numpy as np
import ml_dtypes
from contextlib import ExitStack

import concourse.bass as bass
import concourse.bacc as bacc
import concourse.tile as tile
from concourse import mybir
from concourse.bass_utils import run_bass_kernel_spmd

F32 = mybir.dt.float32
BF16 = mybir.dt.bfloat16
NPBF16 = ml_dtypes.bfloat16
AF = mybir.ActivationFunctionType
ALU = mybir.AluOpType

N_CORES = 8
B_FULL, C, H, W, N = 256, 256, 4, 4, 2048
HW = H * W                  # 16
B = B_FULL // N_CORES       # 32 batches per core
KC = C // 128               # 2 contraction chunks of 128
GB = 4                      # batches per partition group (offsets 0/32/64/96)
NG = B // GB                # 8 groups
NCH = 4                     # n chunks per group chain
NW = N // NCH               # 512 (one PSUM bank)
CONST = 1e-4

# ln(1+t) ~ C1*t + C2*t^2 on [0,1], minimax through origin (max err 4.5e-3)
C1, C2 = 0.94, -0.251
# poly(1) evaluated exactly as the bf16 pipeline does for the zero-pad rows
# (t = exp(0) = 1): v1 = bf16(C2 + C1); c = bf16(v1); num_pad = bf16(c)
POLY1 = float(
    np.float32(NPBF16(np.float32(NPBF16(np.float32(C1 + C2)))))
)
WPAD = CONST / POLY1


def aux_inputs():
    # bd[k, m]: per batch-slot m, weight 1 for its 16 real hw rows and WPAD
    # for its 16 zero-pad rows (whose num is exactly poly(1)), so the bd
    # reduction yields sum_hw softplus + 16*CONST directly
    bd = np.zeros((128, GB), NPBF16)
    for k in range(128):
        bd[k, k // 32] = 1.0 if k % 32 < HW else WPAD
    # exp4[m, p] = 1 iff partition p belongs to batch-slot m's 32-block
    exp4 = np.zeros((GB, 128), NPBF16)
    for p in range(128):
        exp4[p // 32, p] = 1.0
    return {"bd": bd, "exp4": exp4}


def build_nc(debug=False):
    nc = bacc.Bacc(None, target_bir_lowering=False, debug=debug)
    feat = nc.dram_tensor("fpad", [128, KC, HW, B], BF16, kind="ExternalInput")
    ftp = nc.dram_tensor("ftpack", [128, NG, KC, 128], BF16, kind="ExternalInput")
    wts = nc.dram_tensor("weights", [B, C, N], BF16, kind="ExternalInput")
    out = nc.dram_tensor("out", [B, C, N], BF16, kind="ExternalOutput")
    bd_d = nc.dram_tensor("bd", [128, GB], BF16, kind="ExternalInput")
    exp_d = nc.dram_tensor("exp4", [GB, 128], BF16, kind="ExternalInput")

    # [ci, b, kc, nch, nw] views of the DRAM tensors
    wts_r = wts.ap().rearrange("b (kc ci) (nch nw) -> ci b kc nch nw", kc=KC, nch=NCH)
    out_r = out.ap().rearrange("b (kc ci) (nch nw) -> ci b kc nch nw", kc=KC, nch=NCH)

    with tile.TileContext(nc) as tc, ExitStack() as ctx:
        singles = ctx.enter_context(tc.tile_pool(name="singles", bufs=1))
        wpool = ctx.enter_context(tc.tile_pool(name="w", bufs=12))
        opool = ctx.enter_context(tc.tile_pool(name="o", bufs=8))
        numpool = ctx.enter_context(tc.tile_pool(name="num", bufs=6))
        attpool = ctx.enter_context(tc.tile_pool(name="att", bufs=3))
        smallpool = ctx.enter_context(tc.tile_pool(name="small", bufs=3))
        ps_sc = ctx.enter_context(tc.tile_pool(name="ps_sc", bufs=2, space="PSUM"))
        ps_dr = ctx.enter_context(tc.tile_pool(name="ps_dr", bufs=2, space="PSUM"))
        ps_o = ctx.enter_context(tc.tile_pool(name="ps_o", bufs=2, space="PSUM"))

        w_t = {}

        def issue_weights(g):
            for b in range(GB * g, GB * (g + 1)):
                w_t[b] = wpool.tile([128, KC, NCH, NW], BF16, tag="w", name="w_t")
                # two half-batch transfers interleave more smoothly with the
                # output pieces in the DMA FIFO than one 2.9us transfer
                nc.sync.dma_start(out=w_t[b][:, 0], in_=wts_r[:, b, 0])
                nc.sync.dma_start(out=w_t[b][:, 1], in_=wts_r[:, b, 1])

        # features first (mm1's stationary), then the first weight group, then
        # the small/later-needed tensors — keeps the DMA queue dense at start
        # while letting mm1 begin as early as possible
        # h-major features: the 16 real hw rows upload as one contiguous
        # block at full DMA speed; the 16 zero rows come from a Pool memset
        f_t = singles.tile([128, KC, 32, B], BF16, name="f_t")
        nc.gpsimd.memset(f_t, 0.0)
        nc.sync.dma_start(out=f_t[:, :, :HW, :], in_=feat.ap())
        bd_t = singles.tile([128, GB], BF16, name="bd_t")
        nc.sync.dma_start(out=bd_t, in_=bd_d.ap())
        issue_weights(0)
        exp_t = singles.tile([GB, 128], BF16, name="exp_t")
        nc.sync.dma_start(out=exp_t, in_=exp_d.ap())
        # fT[hw, c] per batch at its slot's partition offset (mm2 stationary);
        # slot 3 additionally needs a full-K stationary with zeros outside
        # rows 96..111 (operand base partitions max out at 64) — built from
        # the same upload via a Pool memset plus one 64KB DRAM round trip.
        ftp_t = singles.tile([128, NG, KC, 128], BF16, name="ftp_t")
        nc.sync.dma_start(out=ftp_t, in_=ftp.ap())
        ftp3_t = singles.tile([128, NG, KC, 128], BF16, name="ftp3_t")
        nc.gpsimd.memset(ftp3_t, 0.0)
        nc.sync.dma_start(out=ftp3_t[96 : 96 + HW], in_=ftp.ap()[96 : 96 + HW])

        # copy-engine rotation per [128, 2*NW] pair-copy: GPSIMD cannot read
        # PSUM on real TRN2, so the PSUM->SBUF output copies are split across
        # ACT (x10) and DVE (x6) only; Pool evaluates the softplus polynomial
        COPY_ENG = ["A", "D", "A", "A", "D", "A", "D", "A",
                    "A", "D", "A", "D", "A", "A", "D", "A"]
        state = {}      # g -> dict with num_l, att_t

        def mm1_chunk(g, nb):
            """mm1 for chunk nb of group g + the full softplus chain:
            relu (ACT), |x| = 2*relu - x (DVE), t = exp(-|x|) (ACT),
            c = t*(C1 + C2*t) and num = relu + c (Pool, all-SBUF)."""
            st = state[g]
            sc_ps = ps_sc.tile([128, NW], F32, tag="sc", name="sc_ps")
            for j in range(GB):
                for kc in range(KC):
                    nc.tensor.matmul(
                        sc_ps[32 * j : 32 * j + 32, :],
                        f_t[:, kc, :, GB * g + j],
                        w_t[GB * g + j][:, kc, nb, :],
                        start=(kc == 0),
                        stop=(kc == KC - 1),
                        tile_position=(0, 32 * j),
                    )
            t_relu = numpool.tile([128, NW], BF16, tag="trelu", bufs=9, name="t_relu")
            t_abs = numpool.tile([128, NW], BF16, tag="tabs", bufs=6, name="t_abs")
            t_exp = numpool.tile([128, NW], BF16, tag="texp", bufs=6, name="t_exp")
            t_v1 = numpool.tile([128, NW], BF16, tag="tv1", bufs=4, name="t_v1")
            t_c = numpool.tile([128, NW], BF16, tag="tc", bufs=4, name="t_c")
            num_t = numpool.tile([128, NW], BF16, tag="num", bufs=6, name="num_t")
            with nc.allow_low_precision(reason="bf16 within 2e-2 tolerance"):
                nc.scalar.activation(t_relu, sc_ps, AF.Relu)
                nc.vector.scalar_tensor_tensor(
                    t_abs, t_relu, 2.0, sc_ps, op0=ALU.mult, op1=ALU.subtract
                )
                nc.scalar.activation(t_exp, t_abs, AF.Exp, scale=-1.0)
                nc.gpsimd.tensor_scalar(
                    t_v1, t_exp, C2, C1, op0=ALU.mult, op1=ALU.add
                )
                nc.gpsimd.tensor_mul(t_c, t_v1, t_exp)
                nc.gpsimd.tensor_add(num_t, t_relu, t_c)
            st["num_l"].append(num_t)

        def tail(g):
            """denom/recip/bcast/att for group g (emitted after mm1(g+1,0)
            so the in-order PE queue has num(g) ready)."""
            st = state[g]
            att_t = st["att_t"] = attpool.tile([128, NCH, NW], BF16, name="att_t")
            num_l = st["num_l"]
            for nb in range(NCH):
                d_ps = ps_dr.tile([GB, NW], F32, tag="dr", name="d_ps")
                nc.tensor.matmul(d_ps, bd_t, num_l[nb], start=True, stop=True)
                r_t = smallpool.tile([GB, NW], BF16, tag="rb", name="r_t")
                with nc.allow_low_precision(reason="bf16 within 2e-2 tolerance"):
                    nc.vector.reciprocal(r_t, d_ps)
                rb_ps = ps_dr.tile([128, NW], F32, tag="dr", name="rb_ps")
                nc.tensor.matmul(rb_ps, exp_t, r_t, start=True, stop=True)
                # att = (num + CONST) * (1/denom)
                with nc.allow_low_precision(reason="bf16 within 2e-2 tolerance"):
                    nc.vector.scalar_tensor_tensor(
                        att_t[:, nb, :],
                        num_l[nb],
                        CONST,
                        rb_ps,
                        op0=ALU.add,
                        op1=ALU.mult,
                    )

        def emit_out(g):
            """mm2 + PSUM->SBUF copies + output DMA for group g."""
            st = state[g]
            att_t = st["att_t"]
            rot = COPY_ENG
            # in the drain there are no weight transfers left to hide the
            # mm2->copy->DMA latency, so ship each pair as its own DMA
            split_dma = g >= NG - 2
            ev = 0
            for j in range(GB):
                for kc in range(KC):
                    o_sb = opool.tile([128, NCH, NW], BF16, tag="o", name="o_sb")
                    for nbp in range(NCH // 2):
                        o_ps = ps_o.tile([128, 2, NW], F32, tag="o", name="o_ps")
                        for h in range(2):
                            nb = 2 * nbp + h
                            if j < 3:
                                nc.tensor.matmul(
                                    o_ps[:, h, :],
                                    ftp_t[32 * j : 32 * j + HW, g, kc, :],
                                    att_t[32 * j : 32 * j + HW, nb, :],
                                    start=True,
                                    stop=True,
                                )
                            else:
                                nc.tensor.matmul(
                                    o_ps[:, h, :],
                                    ftp3_t[:, g, kc, :],
                                    att_t[:, nb, :],
                                    start=True,
                                    stop=True,
                                )
                        dst = o_sb[:, 2 * nbp : 2 * nbp + 2, :]
                        eng = rot[ev % 16]
                        ev += 1
                        with nc.allow_low_precision(
                            reason="bf16 within 2e-2 tolerance"
                        ):
                            if eng == "A":
                                nc.scalar.copy(dst, o_ps)
                            else:
                                nc.vector.tensor_copy(dst, o_ps)
                        if split_dma:
                            nc.sync.dma_start(
                                out=out_r[
                                    :, GB * g + j, kc, 2 * nbp : 2 * nbp + 2
                                ],
                                in_=dst,
                            )
                    if not split_dma:
                        nc.sync.dma_start(out=out_r[:, GB * g + j, kc], in_=o_sb)

        # iteration g: PE order is
        #   mm1(g, chunk0) | denom/bcast(g-1) | mm2(g-1) | mm1(g, chunks 1-3)
        # so mm2(g-1) (whose copies feed the output DMAs) starts as early as
        # possible while the denom matmuls still never stall the PE queue.
        def iteration(g):
            if g + 1 < NG:
                issue_weights(g + 1)
            state[g] = {"num_l": []}
            mm1_chunk(g, 0)
            if g > 0:
                tail(g - 1)
                emit_out(g - 1)
            for nb in range(1, NCH):
                mm1_chunk(g, nb)

        for g in range(NG):
            iteration(g)
        tail(NG - 1)

        # epilogue: nothing overlaps the last group's output, so emit all
        # first-half pairs (att chunks 0-1) before the second half and ship
        # each pair as its own DMA the moment its copy lands
        g = NG - 1
        att_t = state[g]["att_t"]
        ev = 0
        for nbp in range(NCH // 2):
            for j in range(GB):
                for kc in range(KC):
                    o_ps = ps_o.tile([128, 2, NW], F32, tag="o", name="o_ps")
                    for h in range(2):
                        nb = 2 * nbp + h
                        if j < 3:
                            nc.tensor.matmul(
                                o_ps[:, h, :],
                                ftp_t[32 * j : 32 * j + HW, g, kc, :],
                                att_t[32 * j : 32 * j + HW, nb, :],
                                start=True,
                                stop=True,
                            )
                        else:
                            nc.tensor.matmul(
                                o_ps[:, h, :],
                                ftp3_t[:, g, kc, :],
                                att_t[:, nb, :],
                                start=True,
                                stop=True,
                            )
                    o2 = opool.tile([128, 2, NW], BF16, tag="olast", bufs=6,
                                    name="o2")
                    eng = ["A", "D"][ev % 2]
                    ev += 1
                    with nc.allow_low_precision(
                        reason="bf16 within 2e-2 tolerance"
                    ):
                        if eng == "A":
                            nc.scalar.copy(o2, o_ps)
                        else:
                            nc.vector.tensor_copy(o2, o_ps)
                    nc.sync.dma_start(
                        out=out_r[:, GB * g + j, kc, 2 * nbp : 2 * nbp + 2],
                        in_=o2,
                    )

    nc.compile()
    return nc


_NC_CACHE = {}


def _get_nc():
    if "nc" not in _NC_CACHE:
        _NC_CACHE["nc"] = build_nc()
    return _NC_CACHE["nc"]


def prep_features(features):
    """[B_FULL, C, H, W] f32 -> (fpad [128, KC, HW, B_FULL],
    ftpack [128, ngrp_total, KC, 128]) both bf16.

    fpad[ci, kc, h, b] = f[b, kc*128+ci, h] (only the 16 real rows; the
    kernel zero-fills rows 16-31 on-device).
    ftpack[32j+h, G, kc, ci] = f[4G+j, kc*128+ci, h]: fT at each slot's
    partition offset (mm2 stationary slices; slot 3's full-K zero-padded
    variant is derived on-device).
    """
    f = np.asarray(features, np.float32).reshape(B_FULL, KC, 128, HW)
    # [b, kc, ci, h] -> [ci, kc, h, b]
    fpad = np.ascontiguousarray(f.transpose(2, 1, 3, 0)).astype(NPBF16)

    ngrp = B_FULL // GB
    fg = f.reshape(ngrp, GB, KC, 128, HW)
    ftp = np.zeros((GB, 32, ngrp, KC, 128), np.float32)
    for j in range(GB):
        # [G, kc, ci, h] -> [h, G, kc, ci]
        ftp[j, :HW] = fg[:, j].transpose(3, 0, 1, 2)
    ftp = np.ascontiguousarray(ftp.reshape(128, ngrp, KC, 128)).astype(NPBF16)
    return fpad, ftp


def run(features, weights, trace=False, **kwargs):
    """Shard over 8 cores, run, gather. Returns (out, BassKernelResults)."""
    fpad, ftp = prep_features(features)
    weights = np.asarray(weights, np.float32).astype(NPBF16)
    aux = aux_inputs()
    nc = _get_nc()
    in_maps = []
    for i in range(N_CORES):
        sl = slice(i * B, (i + 1) * B)
        gsl = slice(i * NG, (i + 1) * NG)
        in_maps.append(
            {
                "fpad": np.ascontiguousarray(fpad[:, :, :, sl]),
                "ftpack": np.ascontiguousarray(ftp[:, gsl]),
                "weights": weights[sl],
                **aux,
            }
        )
    res = run_bass_kernel_spmd(
        nc, in_maps, core_ids=list(range(N_CORES)), trace=trace, **kwargs
    )
    out = np.concatenate([r["out"] for r in res.results], axis=0).astype(np.float32)
    return out, res


def kernel(features, weights):
    out, _ = run(features, weights)
    return out


# revision 96
# speedup vs baseline: 1.0078x; 1.0078x over previous
"""Attentional pooling layer on Trainium2 (Bass/Tile), 8-core batch-parallel.

Reference computation per batch b:
    scores[hw, n] = sum_c f[c, hw] * w[c, n]          (mm1)
    num           = softplus(scores)                  (relu/abs/Exp + quad poly)
    denom[n]      = sum_hw num[hw, n] + 16*CONST      (PE reduce, pad-row trick)
    att[hw, n]    = (num + CONST) / denom[n]          (DVE recip, PE bcast, DVE)
    out[c, n]     = sum_hw f[c, hw] * att[hw, n]      (mm2)

The kernel is DMA-bound (weights in + out out dominate), so both weights and
output travel as bf16 (tolerance is 2e-2; this lands ~4e-3), and all matmuls
take bf16 moving operands.  4 batches are packed per 128-partition group at
32-partition offsets 0/32/64/96 (PE tile_position); mm1 runs M=32 with
zero-padded feature columns so the 16 garbage rows per 32-block hold clean
zeros.  softplus(x) = max(x,0) + ln(1+exp(-|x|)); the ln(1+t) factor is a
minimax quadratic C1*t + C2*t^2 (max err 4.5e-3), so the only table-based
activation is Exp — a single table load for the whole kernel.  |x| is
2*relu(x) - x because an op may read PSUM only once; GPSIMD cannot touch PSUM
at all, so Pool evaluates the (all-SBUF) polynomial and num sum while the
PSUM->SBUF output copies rotate over ACT/DVE.  The denominator's +16*CONST
rides inside the bd reduction matmul: each block's 16 zero-pad rows carry
weight WPAD = CONST/poly(1) instead of 0.

The per-group tail (reduce/broadcast/att/mm2) is software-pipelined one group
behind mm1 so the in-order PE queue never waits on the softplus chain:
PE program is mm1(g, chunk0) | denom+bcast(g-1) | mm2(g-1) | mm1(g, 1-3).

32 batches per core = 8 groups of 4, no ragged tail.
"""

import numpy as np
import ml_dtypes
from contextlib import ExitStack

import concourse.bass as bass
import concourse.bacc as bacc
import concourse.tile as tile
from concourse import mybir
from concourse.bass_utils import run_bass_kernel_spmd

F32 = mybir.dt.float32
BF16 = mybir.dt.bfloat16
NPBF16 = ml_dtypes.bfloat16
AF = mybir.ActivationFunctionType
ALU = mybir.AluOpType

N_CORES = 8
B_FULL, C, H, W, N = 256, 256, 4, 4, 2048
HW = H * W                  # 16
B = B_FULL // N_CORES       # 32 batches per core
KC = C // 128               # 2 contraction chunks of 128
GB = 4                      # batches per partition group (offsets 0/32/64/96)
NG = B // GB                # 8 groups
NCH = 4                     # n chunks per group chain
NW = N // NCH               # 512 (one PSUM bank)
CONST = 1e-4

# ln(1+t) ~ C1*t + C2*t^2 on [0,1], minimax through origin (max err 4.5e-3)
C1, C2 = 0.94, -0.251
# poly(1) evaluated exactly as the bf16 pipeline does for the zero-pad rows
# (t = exp(0) = 1): v1 = bf16(C2 + C1); c = bf16(v1); num_pad = bf16(c)
POLY1 = float(
    np.float32(NPBF16(np.float32(NPBF16(np.float32(C1 + C2)))))
)
WPAD = CONST / POLY1


def aux_inputs():
    # bd[k, m]: per batch-slot m, weight 1 for its 16 real hw rows and WPAD
    # for its 16 zero-pad rows (whose num is exactly poly(1)), so the bd
    # reduction yields sum_hw softplus + 16*CONST directly
    bd = np.zeros((128, GB), NPBF16)
    for k in range(128):
        bd[k, k // 32] = 1.0 if k % 32 < HW else WPAD
    # exp4[m, p] = 1 iff partition p belongs to batch-slot m's 32-block
    exp4 = np.zeros((GB, 128), NPBF16)
    for p in range(128):
        exp4[p // 32, p] = 1.0
    return {"bd": bd, "exp4": exp4}


def build_nc(debug=False):
    nc = bacc.Bacc(None, target_bir_lowering=False, debug=debug)
    feat = nc.dram_tensor("fpad", [128, KC, HW, B], BF16, kind="ExternalInput")
    ftp = nc.dram_tensor("ftpack", [128, NG, KC, 128], BF16, kind="ExternalInput")
    wts = nc.dram_tensor("weights", [B, C, N], BF16, kind="ExternalInput")
    out = nc.dram_tensor("out", [B, C, N], BF16, kind="ExternalOutput")
    bd_d = nc.dram_tensor("bd", [128, GB], BF16, kind="ExternalInput")
    exp_d = nc.dram_tensor("exp4", [GB, 128], BF16, kind="ExternalInput")

    # [ci, b, kc, nch, nw] views of the DRAM tensors
    wts_r = wts.ap().rearrange("b (kc ci) (nch nw) -> ci b kc nch nw", kc=KC, nch=NCH)
    out_r = out.ap().rearrange("b (kc ci) (nch nw) -> ci b kc nch nw", kc=KC, nch=NCH)

    with tile.TileContext(nc) as tc, ExitStack() as ctx:
        singles = ctx.enter_context(tc.tile_pool(name="singles", bufs=1))
        wpool = ctx.enter_context(tc.tile_pool(name="w", bufs=12))
        opool = ctx.enter_context(tc.tile_pool(name="o", bufs=8))
        numpool = ctx.enter_context(tc.tile_pool(name="num", bufs=6))
        attpool = ctx.enter_context(tc.tile_pool(name="att", bufs=3))
        smallpool = ctx.enter_context(tc.tile_pool(name="small", bufs=3))
        ps_sc = ctx.enter_context(tc.tile_pool(name="ps_sc", bufs=2, space="PSUM"))
        ps_dr = ctx.enter_context(tc.tile_pool(name="ps_dr", bufs=2, space="PSUM"))
        ps_o = ctx.enter_context(tc.tile_pool(name="ps_o", bufs=2, space="PSUM"))

        w_t = {}

        def issue_weights(g):
            for b in range(GB * g, GB * (g + 1)):
                w_t[b] = wpool.tile([128, KC, NCH, NW], BF16, tag="w", name="w_t")
                # two half-batch transfers interleave more smoothly with the
                # output pieces in the DMA FIFO than one 2.9us transfer
                nc.sync.dma_start(out=w_t[b][:, 0], in_=wts_r[:, b, 0])
                nc.sync.dma_start(out=w_t[b][:, 1], in_=wts_r[:, b, 1])

        # features first (mm1's stationary), then the first weight group, then
        # the small/later-needed tensors — keeps the DMA queue dense at start
        # while letting mm1 begin as early as possible
        # h-major features: the 16 real hw rows upload as one contiguous
        # block at full DMA speed; the 16 zero rows come from a Pool memset
        f_t = singles.tile([128, KC, 32, B], BF16, name="f_t")
        nc.gpsimd.memset(f_t, 0.0)
        nc.sync.dma_start(out=f_t[:, :, :HW, :], in_=feat.ap())
        bd_t = singles.tile([128, GB], BF16, name="bd_t")
        nc.sync.dma_start(out=bd_t, in_=bd_d.ap())
        issue_weights(0)
        exp_t = singles.tile([GB, 128], BF16, name="exp_t")
        nc.sync.dma_start(out=exp_t, in_=exp_d.ap())
        # fT[hw, c] per batch at its slot's partition offset (mm2 stationary);
        # slot 3 additionally needs a full-K stationary with zeros outside
        # rows 96..111 (operand base partitions max out at 64) — built from
        # the same upload via a Pool memset plus one 64KB DRAM round trip.
        ftp_t = singles.tile([128, NG, KC, 128], BF16, name="ftp_t")
        nc.sync.dma_start(out=ftp_t, in_=ftp.ap())
        ftp3_t = singles.tile([128, NG, KC, 128], BF16, name="ftp3_t")
        nc.gpsimd.memset(ftp3_t, 0.0)
        nc.sync.dma_start(out=ftp3_t[96 : 96 + HW], in_=ftp.ap()[96 : 96 + HW])

        # copy-engine rotation per [128, 2*NW] pair-copy: GPSIMD cannot read
        # PSUM on real TRN2, so the PSUM->SBUF output copies are split across
        # ACT (x10) and DVE (x6) only; Pool evaluates the softplus polynomial
        COPY_ENG = ["A", "D", "A", "A", "D", "A", "D", "A",
                    "A", "D", "A", "D", "A", "A", "D", "A"]
        state = {}      # g -> dict with num_l, att_t

        def mm1_chunk(g, nb):
            """mm1 for chunk nb of group g + the full softplus chain:
            relu (ACT), |x| = 2*relu - x (DVE), t = exp(-|x|) (ACT),
            c = t*(C1 + C2*t) and num = relu + c (Pool, all-SBUF)."""
            st = state[g]
            sc_ps = ps_sc.tile([128, NW], F32, tag="sc", name="sc_ps")
            for j in range(GB):
                for kc in range(KC):
                    nc.tensor.matmul(
                        sc_ps[32 * j : 32 * j + 32, :],
                        f_t[:, kc, :, GB * g + j],
                        w_t[GB * g + j][:, kc, nb, :],
                        start=(kc == 0),
                        stop=(kc == KC - 1),
                        tile_position=(0, 32 * j),
                    )
            t_relu = numpool.tile([128, NW], BF16, tag="trelu", bufs=9, name="t_relu")
            t_abs = numpool.tile([128, NW], BF16, tag="tabs", bufs=6, name="t_abs")
            t_exp = numpool.tile([128, NW], BF16, tag="texp", bufs=6, name="t_exp")
            t_v1 = numpool.tile([128, NW], BF16, tag="tv1", bufs=4, name="t_v1")
            t_c = numpool.tile([128, NW], BF16, tag="tc", bufs=4, name="t_c")
            num_t = numpool.tile([128, NW], BF16, tag="num", bufs=6, name="num_t")
            with nc.allow_low_precision(reason="bf16 within 2e-2 tolerance"):
                nc.scalar.activation(t_relu, sc_ps, AF.Relu)
                nc.vector.scalar_tensor_tensor(
                    t_abs, t_relu, 2.0, sc_ps, op0=ALU.mult, op1=ALU.subtract
                )
                nc.scalar.activation(t_exp, t_abs, AF.Exp, scale=-1.0)
                nc.gpsimd.tensor_scalar(
                    t_v1, t_exp, C2, C1, op0=ALU.mult, op1=ALU.add
                )
                nc.gpsimd.tensor_mul(t_c, t_v1, t_exp)
                nc.gpsimd.tensor_add(num_t, t_relu, t_c)
            st["num_l"].append(num_t)

        def tail(g):
            """denom/recip/bcast/att for group g (emitted after mm1(g+1,0)
            so the in-order PE queue has num(g) ready)."""
            st = state[g]
            att_t = st["att_t"] = attpool.tile([128, NCH, NW], BF16, name="att_t")
            num_l = st["num_l"]
            for nb in range(NCH):
                d_ps = ps_dr.tile([GB, NW], F32, tag="dr", name="d_ps")
                nc.tensor.matmul(d_ps, bd_t, num_l[nb], start=True, stop=True)
                r_t = smallpool.tile([GB, NW], BF16, tag="rb", name="r_t")
                with nc.allow_low_precision(reason="bf16 within 2e-2 tolerance"):
                    nc.vector.reciprocal(r_t, d_ps)
                rb_ps = ps_dr.tile([128, NW], F32, tag="dr", name="rb_ps")
                nc.tensor.matmul(rb_ps, exp_t, r_t, start=True, stop=True)
                # att = (num + CONST) * (1/denom)
                with nc.allow_low_precision(reason="bf16 within 2e-2 tolerance"):
                    nc.vector.scalar_tensor_tensor(
                        att_t[:, nb, :],
                        num_l[nb],
                        CONST,
                        rb_ps,
                        op0=ALU.add,
                        op1=ALU.mult,
                    )

        def emit_out(g):
            """mm2 + PSUM->SBUF copies + output DMA for group g."""
            st = state[g]
            att_t = st["att_t"]
            rot = COPY_ENG
            # in the drain there are no weight transfers left to hide the
            # mm2->copy->DMA latency, so ship each pair as its own DMA
            split_dma = g >= NG - 2
            ev = 0
            for j in range(GB):
                for kc in range(KC):
                    o_sb = opool.tile([128, NCH, NW], BF16, tag="o", name="o_sb")
                    for nbp in range(NCH // 2):
                        o_ps = ps_o.tile([128, 2, NW], F32, tag="o", name="o_ps")
                        for h in range(2):
                            nb = 2 * nbp + h
                            if j < 3:
                                nc.tensor.matmul(
                                    o_ps[:, h, :],
                                    ftp_t[32 * j : 32 * j + HW, g, kc, :],
                                    att_t[32 * j : 32 * j + HW, nb, :],
                                    start=True,
                                    stop=True,
                                )
                            else:
                                nc.tensor.matmul(
                                    o_ps[:, h, :],
                                    ftp3_t[:, g, kc, :],
                                    att_t[:, nb, :],
                                    start=True,
                                    stop=True,
                                )
                        dst = o_sb[:, 2 * nbp : 2 * nbp + 2, :]
                        eng = rot[ev % 16]
                        ev += 1
                        with nc.allow_low_precision(
                            reason="bf16 within 2e-2 tolerance"
                        ):
                            if eng == "A":
                                nc.scalar.copy(dst, o_ps)
                            else:
                                nc.vector.tensor_copy(dst, o_ps)
                        if split_dma:
                            nc.sync.dma_start(
                                out=out_r[
                                    :, GB * g + j, kc, 2 * nbp : 2 * nbp + 2
                                ],
                                in_=dst,
                            )
                    if not split_dma:
                        nc.sync.dma_start(out=out_r[:, GB * g + j, kc], in_=o_sb)

        # iteration g: PE order is
        #   mm1(g, chunk0) | denom/bcast(g-1) | mm2(g-1) | mm1(g, chunks 1-3)
        # so mm2(g-1) (whose copies feed the output DMAs) starts as early as
        # possible while the denom matmuls still never stall the PE queue.
        def iteration(g):
            if g + 1 < NG:
                issue_weights(g + 1)
            state[g] = {"num_l": []}
            mm1_chunk(g, 0)
            if g > 0:
                tail(g - 1)
                emit_out(g - 1)
            for nb in range(1, NCH):
                mm1_chunk(g, nb)

        for g in range(NG):
            iteration(g)
        tail(NG - 1)

        # epilogue: nothing overlaps the last group's output, so emit all
        # first-half pairs (att chunks 0-1) before the second half and ship
        # each pair as its own DMA the moment its copy lands
        g = NG - 1
        att_t = state[g]["att_t"]
        for nbp in range(NCH // 2):
            for j in range(GB):
                for kc in range(KC):
                    o2 = opool.tile([128, 2, NW], BF16, tag="olast", bufs=6,
                                    name="o2")
                    for h in range(2):
                        nb = 2 * nbp + h
                        # mm1 is finished, so ps_sc's banks join ps_o to give
                        # four single-bank slots; ACT and DVE each copy one
                        # half of every pair in parallel
                        pool = ps_sc if h == 0 else ps_o
                        o_ps = pool.tile([128, NW], F32,
                                         tag="sc" if h == 0 else "o",
                                         name="o_ps")
                        if j < 3:
                            nc.tensor.matmul(
                                o_ps,
                                ftp_t[32 * j : 32 * j + HW, g, kc, :],
                                att_t[32 * j : 32 * j + HW, nb, :],
                                start=True,
                                stop=True,
                            )
                        else:
                            nc.tensor.matmul(
                                o_ps,
                                ftp3_t[:, g, kc, :],
                                att_t[:, nb, :],
                                start=True,
                                stop=True,
                            )
                        with nc.allow_low_precision(
                            reason="bf16 within 2e-2 tolerance"
                        ):
                            if h == 0:
                                nc.scalar.copy(o2[:, 0, :], o_ps)
                            else:
                                nc.vector.tensor_copy(o2[:, 1, :], o_ps)
                    nc.sync.dma_start(
                        out=out_r[:, GB * g + j, kc, 2 * nbp : 2 * nbp + 2],
                        in_=o2,
                    )

    nc.compile()
    return nc


_NC_CACHE = {}


def _get_nc():
    if "nc" not in _NC_CACHE:
        _NC_CACHE["nc"] = build_nc()
    return _NC_CACHE["nc"]


def prep_features(features):
    """[B_FULL, C, H, W] f32 -> (fpad [128, KC, HW, B_FULL],
    ftpack [128, ngrp_total, KC, 128]) both bf16.

    fpad[ci, kc, h, b] = f[b, kc*128+ci, h] (only the 16 real rows; the
    kernel zero-fills rows 16-31 on-device).
    ftpack[32j+h, G, kc, ci] = f[4G+j, kc*128+ci, h]: fT at each slot's
    partition offset (mm2 stationary slices; slot 3's full-K zero-padded
    variant is derived on-device).
    """
    f = np.asarray(features, np.float32).reshape(B_FULL, KC, 128, HW)
    # [b, kc, ci, h] -> [ci, kc, h, b]
    fpad = np.ascontiguousarray(f.transpose(2, 1, 3, 0)).astype(NPBF16)

    ngrp = B_FULL // GB
    fg = f.reshape(ngrp, GB, KC, 128, HW)
    ftp = np.zeros((GB, 32, ngrp, KC, 128), np.float32)
    for j in range(GB):
        # [G, kc, ci, h] -> [h, G, kc, ci]
        ftp[j, :HW] = fg[:, j].transpose(3, 0, 1, 2)
    ftp = np.ascontiguousarray(ftp.reshape(128, ngrp, KC, 128)).astype(NPBF16)
    return fpad, ftp


def run(features, weights, trace=False, **kwargs):
    """Shard over 8 cores, run, gather. Returns (out, BassKernelResults)."""
    fpad, ftp = prep_features(features)
    weights = np.asarray(weights, np.float32).astype(NPBF16)
    aux = aux_inputs()
    nc = _get_nc()
    in_maps = []
    for i in range(N_CORES):
        sl = slice(i * B, (i + 1) * B)
        gsl = slice(i * NG, (i + 1) * NG)
        in_maps.append(
            {
                "fpad": np.ascontiguousarray(fpad[:, :, :, sl]),
                "ftpack": np.ascontiguousarray(ftp[:, gsl]),
                "weights": weights[sl],
                **aux,
            }
        )
    res = run_bass_kernel_spmd(
        nc, in_maps, core_ids=list(range(N_CORES)), trace=trace, **kwargs
    )
    out = np.concatenate([r["out"] for r in res.results], axis=0).astype(np.float32)
    return out, res


def kernel(features, weights):
    out, _ = run(features, weights)
    return out


# revision 99
# speedup vs baseline: 1.0081x; 1.0003x over previous
"""Attentional pooling layer on Trainium2 (Bass/Tile), 8-core batch-parallel.

Reference computation per batch b:
    scores[hw, n] = sum_c f[c, hw] * w[c, n]          (mm1)
    num           = softplus(scores)                  (relu/abs/Exp + quad poly)
    denom[n]      = sum_hw num[hw, n] + 16*CONST      (PE reduce, pad-row trick)
    att[hw, n]    = (num + CONST) / denom[n]          (DVE recip, PE bcast, DVE)
    out[c, n]     = sum_hw f[c, hw] * att[hw, n]      (mm2)

The kernel is DMA-bound (weights in + out out dominate), so both weights and
output travel as bf16 (tolerance is 2e-2; this lands ~4e-3), and all matmuls
take bf16 moving operands.  4 batches are packed per 128-partition group at
32-partition offsets 0/32/64/96 (PE tile_position); mm1 runs M=32 with
zero-padded feature columns so the 16 garbage rows per 32-block hold clean
zeros.  softplus(x) = max(x,0) + ln(1+exp(-|x|)); the ln(1+t) factor is a
minimax quadratic C1*t + C2*t^2 (max err 4.5e-3), so the only table-based
activation is Exp — a single table load for the whole kernel.  |x| is
2*relu(x) - x because an op may read PSUM only once; GPSIMD cannot touch PSUM
at all, so Pool evaluates the (all-SBUF) polynomial and num sum while the
PSUM->SBUF output copies rotate over ACT/DVE.  The denominator's +16*CONST
rides inside the bd reduction matmul: each block's 16 zero-pad rows carry
weight WPAD = CONST/poly(1) instead of 0.

The per-group tail (reduce/broadcast/att/mm2) is software-pipelined one group
behind mm1 so the in-order PE queue never waits on the softplus chain:
PE program is mm1(g, chunk0) | denom+bcast(g-1) | mm2(g-1) | mm1(g, 1-3).

32 batches per core = 8 groups of 4, no ragged tail.
"""

import numpy as np
import ml_dtypes
from contextlib import ExitStack

import concourse.bass as bass
import concourse.bacc as bacc
import concourse.tile as tile
from concourse import mybir
from concourse.bass_utils import run_bass_kernel_spmd

F32 = mybir.dt.float32
BF16 = mybir.dt.bfloat16
NPBF16 = ml_dtypes.bfloat16
AF = mybir.ActivationFunctionType
ALU = mybir.AluOpType

N_CORES = 8
B_FULL, C, H, W, N = 256, 256, 4, 4, 2048
HW = H * W                  # 16
B = B_FULL // N_CORES       # 32 batches per core
KC = C // 128               # 2 contraction chunks of 128
GB = 4                      # batches per partition group (offsets 0/32/64/96)
NG = B // GB                # 8 groups
NCH = 4                     # n chunks per group chain
NW = N // NCH               # 512 (one PSUM bank)
CONST = 1e-4

# ln(1+t) ~ C1*t + C2*t^2 on [0,1], minimax through origin (max err 4.5e-3)
C1, C2 = 0.94, -0.251
# poly(1) evaluated exactly as the bf16 pipeline does for the zero-pad rows
# (t = exp(0) = 1): v1 = bf16(C2 + C1); c = bf16(v1); num_pad = bf16(c)
POLY1 = float(
    np.float32(NPBF16(np.float32(NPBF16(np.float32(C1 + C2)))))
)
WPAD = CONST / POLY1


def aux_inputs():
    # bd[k, m]: per batch-slot m, weight 1 for its 16 real hw rows and WPAD
    # for its 16 zero-pad rows (whose num is exactly poly(1)), so the bd
    # reduction yields sum_hw softplus + 16*CONST directly
    bd = np.zeros((128, GB), NPBF16)
    for k in range(128):
        bd[k, k // 32] = 1.0 if k % 32 < HW else WPAD
    # exp4[m, p] = 1 iff partition p belongs to batch-slot m's 32-block
    exp4 = np.zeros((GB, 128), NPBF16)
    for p in range(128):
        exp4[p // 32, p] = 1.0
    return {"bd": bd, "exp4": exp4}


def build_nc(debug=False):
    nc = bacc.Bacc(None, target_bir_lowering=False, debug=debug)
    feat = nc.dram_tensor("fpad", [128, KC, HW, B], BF16, kind="ExternalInput")
    ftp = nc.dram_tensor("ftpack", [128, NG, KC, 128], BF16, kind="ExternalInput")
    wts = nc.dram_tensor("weights", [B, C, N], BF16, kind="ExternalInput")
    out = nc.dram_tensor("out", [B, C, N], BF16, kind="ExternalOutput")
    bd_d = nc.dram_tensor("bd", [128, GB], BF16, kind="ExternalInput")
    exp_d = nc.dram_tensor("exp4", [GB, 128], BF16, kind="ExternalInput")

    # [ci, b, kc, nch, nw] views of the DRAM tensors
    wts_r = wts.ap().rearrange("b (kc ci) (nch nw) -> ci b kc nch nw", kc=KC, nch=NCH)
    out_r = out.ap().rearrange("b (kc ci) (nch nw) -> ci b kc nch nw", kc=KC, nch=NCH)

    with tile.TileContext(nc) as tc, ExitStack() as ctx:
        singles = ctx.enter_context(tc.tile_pool(name="singles", bufs=1))
        wpool = ctx.enter_context(tc.tile_pool(name="w", bufs=12))
        opool = ctx.enter_context(tc.tile_pool(name="o", bufs=8))
        numpool = ctx.enter_context(tc.tile_pool(name="num", bufs=6))
        attpool = ctx.enter_context(tc.tile_pool(name="att", bufs=3))
        smallpool = ctx.enter_context(tc.tile_pool(name="small", bufs=3))
        ps_sc = ctx.enter_context(tc.tile_pool(name="ps_sc", bufs=2, space="PSUM"))
        ps_dr = ctx.enter_context(tc.tile_pool(name="ps_dr", bufs=2, space="PSUM"))
        ps_o = ctx.enter_context(tc.tile_pool(name="ps_o", bufs=2, space="PSUM"))

        w_t = {}

        def issue_weights(g):
            for b in range(GB * g, GB * (g + 1)):
                w_t[b] = wpool.tile([128, KC, NCH, NW], BF16, tag="w", name="w_t")
                # two half-batch transfers interleave more smoothly with the
                # output pieces in the DMA FIFO than one 2.9us transfer
                nc.sync.dma_start(out=w_t[b][:, 0], in_=wts_r[:, b, 0])
                nc.sync.dma_start(out=w_t[b][:, 1], in_=wts_r[:, b, 1])

        # features first (mm1's stationary), then the first weight group, then
        # the small/later-needed tensors — keeps the DMA queue dense at start
        # while letting mm1 begin as early as possible
        # h-major features: the 16 real hw rows upload as one contiguous
        # block at full DMA speed; the 16 zero rows come from a Pool memset
        f_t = singles.tile([128, KC, 32, B], BF16, name="f_t")
        nc.gpsimd.memset(f_t, 0.0)
        nc.sync.dma_start(out=f_t[:, :, :HW, :], in_=feat.ap())
        bd_t = singles.tile([128, GB], BF16, name="bd_t")
        nc.sync.dma_start(out=bd_t, in_=bd_d.ap())
        issue_weights(0)
        exp_t = singles.tile([GB, 128], BF16, name="exp_t")
        nc.sync.dma_start(out=exp_t, in_=exp_d.ap())
        # fT[hw, c] per batch at its slot's partition offset (mm2 stationary);
        # slot 3 additionally needs a full-K stationary with zeros outside
        # rows 96..111 (operand base partitions max out at 64) — built from
        # the same upload via a Pool memset plus one 64KB DRAM round trip.
        ftp_t = singles.tile([128, NG, KC, 128], BF16, name="ftp_t")
        nc.sync.dma_start(out=ftp_t, in_=ftp.ap())
        ftp3_t = singles.tile([128, NG, KC, 128], BF16, name="ftp3_t")
        nc.gpsimd.memset(ftp3_t, 0.0)
        nc.sync.dma_start(out=ftp3_t[96 : 96 + HW], in_=ftp.ap()[96 : 96 + HW])

        # copy-engine rotation per [128, 2*NW] pair-copy: GPSIMD cannot read
        # PSUM on real TRN2, so the PSUM->SBUF output copies are split across
        # ACT (x10) and DVE (x6) only; Pool evaluates the softplus polynomial
        COPY_ENG = ["A", "D", "A", "A", "D", "A", "D", "A",
                    "A", "D", "A", "D", "A", "A", "D", "A"]
        state = {}      # g -> dict with num_l, att_t

        def mm1_chunk(g, nb):
            """mm1 for chunk nb of group g + the full softplus chain:
            relu (ACT), |x| = 2*relu - x (DVE), t = exp(-|x|) (ACT),
            c = t*(C1 + C2*t) and num = relu + c (Pool, all-SBUF)."""
            st = state[g]
            sc_ps = ps_sc.tile([128, NW], F32, tag="sc", name="sc_ps")
            for j in range(GB):
                for kc in range(KC):
                    nc.tensor.matmul(
                        sc_ps[32 * j : 32 * j + 32, :],
                        f_t[:, kc, :, GB * g + j],
                        w_t[GB * g + j][:, kc, nb, :],
                        start=(kc == 0),
                        stop=(kc == KC - 1),
                        tile_position=(0, 32 * j),
                    )
            t_relu = numpool.tile([128, NW], BF16, tag="trelu", bufs=9, name="t_relu")
            t_abs = numpool.tile([128, NW], BF16, tag="tabs", bufs=6, name="t_abs")
            t_exp = numpool.tile([128, NW], BF16, tag="texp", bufs=6, name="t_exp")
            t_v1 = numpool.tile([128, NW], BF16, tag="tv1", bufs=4, name="t_v1")
            t_c = numpool.tile([128, NW], BF16, tag="tc", bufs=4, name="t_c")
            num_t = numpool.tile([128, NW], BF16, tag="num", bufs=6, name="num_t")
            with nc.allow_low_precision(reason="bf16 within 2e-2 tolerance"):
                nc.scalar.activation(t_relu, sc_ps, AF.Relu)
                nc.vector.scalar_tensor_tensor(
                    t_abs, t_relu, 2.0, sc_ps, op0=ALU.mult, op1=ALU.subtract
                )
                nc.scalar.activation(t_exp, t_abs, AF.Exp, scale=-1.0)
                nc.gpsimd.tensor_scalar(
                    t_v1, t_exp, C2, C1, op0=ALU.mult, op1=ALU.add
                )
                nc.gpsimd.tensor_mul(t_c, t_v1, t_exp)
                nc.gpsimd.tensor_add(num_t, t_relu, t_c)
            st["num_l"].append(num_t)

        def tail(g):
            """denom/recip/bcast/att for group g (emitted after mm1(g+1,0)
            so the in-order PE queue has num(g) ready)."""
            st = state[g]
            att_t = st["att_t"] = attpool.tile([128, NCH, NW], BF16, name="att_t")
            num_l = st["num_l"]
            for nb in range(NCH):
                d_ps = ps_dr.tile([GB, NW], F32, tag="dr", name="d_ps")
                nc.tensor.matmul(d_ps, bd_t, num_l[nb], start=True, stop=True)
                r_t = smallpool.tile([GB, NW], BF16, tag="rb", name="r_t")
                with nc.allow_low_precision(reason="bf16 within 2e-2 tolerance"):
                    nc.vector.reciprocal(r_t, d_ps)
                rb_ps = ps_dr.tile([128, NW], F32, tag="dr", name="rb_ps")
                nc.tensor.matmul(rb_ps, exp_t, r_t, start=True, stop=True)
                # att = (num + CONST) * (1/denom)
                with nc.allow_low_precision(reason="bf16 within 2e-2 tolerance"):
                    nc.vector.scalar_tensor_tensor(
                        att_t[:, nb, :],
                        num_l[nb],
                        CONST,
                        rb_ps,
                        op0=ALU.add,
                        op1=ALU.mult,
                    )

        def emit_out(g):
            """mm2 + PSUM->SBUF copies + output DMA for group g."""
            st = state[g]
            att_t = st["att_t"]
            rot = COPY_ENG
            # in the drain there are no weight transfers left to hide the
            # mm2->copy->DMA latency, so ship each pair as its own DMA
            split_dma = g >= NG - 2
            ev = 0
            for j in range(GB):
                for kc in range(KC):
                    o_sb = opool.tile([128, NCH, NW], BF16, tag="o", name="o_sb")
                    for nbp in range(NCH // 2):
                        o_ps = ps_o.tile([128, 2, NW], F32, tag="o", name="o_ps")
                        for h in range(2):
                            nb = 2 * nbp + h
                            if j < 3:
                                nc.tensor.matmul(
                                    o_ps[:, h, :],
                                    ftp_t[32 * j : 32 * j + HW, g, kc, :],
                                    att_t[32 * j : 32 * j + HW, nb, :],
                                    start=True,
                                    stop=True,
                                )
                            else:
                                nc.tensor.matmul(
                                    o_ps[:, h, :],
                                    ftp3_t[:, g, kc, :],
                                    att_t[:, nb, :],
                                    start=True,
                                    stop=True,
                                )
                        dst = o_sb[:, 2 * nbp : 2 * nbp + 2, :]
                        eng = rot[ev % 16]
                        ev += 1
                        with nc.allow_low_precision(
                            reason="bf16 within 2e-2 tolerance"
                        ):
                            if eng == "A":
                                nc.scalar.copy(dst, o_ps)
                            else:
                                nc.vector.tensor_copy(dst, o_ps)
                        if split_dma:
                            nc.sync.dma_start(
                                out=out_r[
                                    :, GB * g + j, kc, 2 * nbp : 2 * nbp + 2
                                ],
                                in_=dst,
                            )
                    if not split_dma:
                        nc.sync.dma_start(out=out_r[:, GB * g + j, kc], in_=o_sb)

        # iteration g: PE order is
        #   mm1(g, chunk0) | denom/bcast(g-1) | mm2(g-1) | mm1(g, chunks 1-3)
        # so mm2(g-1) (whose copies feed the output DMAs) starts as early as
        # possible while the denom matmuls still never stall the PE queue.
        def iteration(g):
            if g + 1 < NG:
                issue_weights(g + 1)
            state[g] = {"num_l": []}
            mm1_chunk(g, 0)
            if g > 0:
                tail(g - 1)
                emit_out(g - 1)
            for nb in range(1, NCH):
                mm1_chunk(g, nb)

        for g in range(NG):
            iteration(g)
        tail(NG - 1)

        # epilogue: nothing overlaps the last group's output, so emit all
        # first-half pairs (att chunks 0-1) before the second half and ship
        # each pair as its own DMA the moment its copy lands
        g = NG - 1
        att_t = state[g]["att_t"]
        for nbp in range(NCH // 2):
            for j in range(GB):
                for kc in range(KC):
                    o2 = opool.tile([128, 2, NW], BF16, tag="olast", bufs=6,
                                    name="o2")
                    for h in range(2):
                        nb = 2 * nbp + h
                        # mm1 is finished, so ps_sc's banks join ps_o to give
                        # four single-bank slots; ACT and DVE each copy one
                        # half of every pair in parallel
                        pool = ps_sc if h == 0 else ps_o
                        o_ps = pool.tile([128, NW], F32,
                                         tag="sc" if h == 0 else "o",
                                         name="o_ps")
                        if j < 3:
                            nc.tensor.matmul(
                                o_ps,
                                ftp_t[32 * j : 32 * j + HW, g, kc, :],
                                att_t[32 * j : 32 * j + HW, nb, :],
                                start=True,
                                stop=True,
                            )
                        else:
                            nc.tensor.matmul(
                                o_ps,
                                ftp3_t[:, g, kc, :],
                                att_t[:, nb, :],
                                start=True,
                                stop=True,
                            )
                        with nc.allow_low_precision(
                            reason="bf16 within 2e-2 tolerance"
                        ):
                            if h == 0:
                                nc.scalar.copy(o2[:, 0, :], o_ps)
                            else:
                                nc.vector.tensor_copy(o2[:, 1, :], o_ps)
                    if nbp == 1 and j == GB - 1 and kc == KC - 1:
                        # ship the final piece as two half transfers so the
                        # last one starts (and the kernel ends) earlier
                        for h in range(2):
                            nc.sync.dma_start(
                                out=out_r[:, GB * g + j, kc, 2 * nbp + h],
                                in_=o2[:, h, :],
                            )
                    else:
                        nc.sync.dma_start(
                            out=out_r[:, GB * g + j, kc, 2 * nbp : 2 * nbp + 2],
                            in_=o2,
                        )

    nc.compile()
    return nc


_NC_CACHE = {}


def _get_nc():
    if "nc" not in _NC_CACHE:
        _NC_CACHE["nc"] = build_nc()
    return _NC_CACHE["nc"]


def prep_features(features):
    """[B_FULL, C, H, W] f32 -> (fpad [128, KC, HW, B_FULL],
    ftpack [128, ngrp_total, KC, 128]) both bf16.

    fpad[ci, kc, h, b] = f[b, kc*128+ci, h] (only the 16 real rows; the
    kernel zero-fills rows 16-31 on-device).
    ftpack[32j+h, G, kc, ci] = f[4G+j, kc*128+ci, h]: fT at each slot's
    partition offset (mm2 stationary slices; slot 3's full-K zero-padded
    variant is derived on-device).
    """
    f = np.asarray(features, np.float32).reshape(B_FULL, KC, 128, HW)
    # [b, kc, ci, h] -> [ci, kc, h, b]
    fpad = np.ascontiguousarray(f.transpose(2, 1, 3, 0)).astype(NPBF16)

    ngrp = B_FULL // GB
    fg = f.reshape(ngrp, GB, KC, 128, HW)
    ftp = np.zeros((GB, 32, ngrp, KC, 128), np.float32)
    for j in range(GB):
        # [G, kc, ci, h] -> [h, G, kc, ci]
        ftp[j, :HW] = fg[:, j].transpose(3, 0, 1, 2)
    ftp = np.ascontiguousarray(ftp.reshape(128, ngrp, KC, 128)).astype(NPBF16)
    return fpad, ftp


def run(features, weights, trace=False, **kwargs):
    """Shard over 8 cores, run, gather. Returns (out, BassKernelResults)."""
    fpad, ftp = prep_features(features)
    weights = np.asarray(weights, np.float32).astype(NPBF16)
    aux = aux_inputs()
    nc = _get_nc()
    in_maps = []
    for i in range(N_CORES):
        sl = slice(i * B, (i + 1) * B)
        gsl = slice(i * NG, (i + 1) * NG)
        in_maps.append(
            {
                "fpad": np.ascontiguousarray(fpad[:, :, :, sl]),
                "ftpack": np.ascontiguousarray(ftp[:, gsl]),
                "weights": weights[sl],
                **aux,
            }
        )
    res = run_bass_kernel_spmd(
        nc, in_maps, core_ids=list(range(N_CORES)), trace=trace, **kwargs
    )
    out = np.concatenate([r["out"] for r in res.results], axis=0).astype(np.float32)
    return out, res


def kernel(features, weights):
    out, _ = run(features, weights)
    return out
